# revision 17
# baseline (speedup 1.0000x reference)
"""Trainium2 Bass kernel for nn_Attention_927712935992.

Fused LayerNorm + QKV projection + masked softmax attention + output
projection, sharded over 8 NeuronCores: core c handles batch c//4 and
heads [4*(c%4), 4*(c%4)+4) of 16.  Weights are replicated (sliced per
core); the (B*H, L, L) score tensor is split along its first axis.

Per-core dataflow (all matmul operands bf16, fp32 PSUM accumulation):
  - host pre-transposes x -> xT (D, L) and the weight slices
  - device: token mean via ones-matmul rows; var/rstd from mu/musq rows
  - LN is folded algebraically into the QKV matmul:
      q = rstd * [ (x @ W'^T) - mu * rowsum(W') ],  W' = ln_w * W
    using an augmented K=1 contraction row (mu) with lhsT = -rowsum(W').
  - scores^T (keys on partitions) per (head, qtile): 2-head row-packed
    K=64 matmuls; exp on ScalarE (scale=1/8) straight from PSUM.
  - masking is free: v rows and the appended ones-column of v are
    multiplied by the 0/1 key mask, so masked keys drop out of both the
    context accumulation and the softmax denominator.
  - context^T = [v|mask] ^T E^T with a rowsum row; normalize by the
    reciprocal rowsum (gpsimd partition-broadcast) into bf16 context.
  - out partial = context^T.T @ out_w_slice^T accumulated over 4 heads.
Host sums the 4 per-batch partials and adds out_b.
"""

import numpy as np

import concourse.bass as bass
import concourse.tile as tile
from concourse import bacc
from concourse import mybir
from concourse import bass_utils
from concourse.masks import make_identity

L, B, D, H, HD = 2048, 2, 1024, 16, 64
NCORES = 8
HEADS_PER_CORE = 4
KC = D // 128            # 8 contraction chunks of 128
TT = 4                   # query tiles of 512
TK = L // 128            # 16 key/token chunks of 128
F32 = mybir.dt.float32
BF16 = mybir.dt.bfloat16
SCALE = 0.125            # 1/sqrt(64)
LN_EPS = 1e-12

_CACHE = {}


def _build_nc():
    nc = bacc.Bacc("TRN2", target_bir_lowering=False, debug=False)

    xT_d = nc.dram_tensor("xT", [D, L], F32, kind="ExternalInput").ap()
    wT_d = nc.dram_tensor("wT", [D, 768], F32, kind="ExternalInput").ap()
    negc_d = nc.dram_tensor("negc", [1, 768], F32, kind="ExternalInput").ap()
    owT_d = nc.dram_tensor("owT", [64, 4, D], F32, kind="ExternalInput").ap()
    maskf_d = nc.dram_tensor("maskf", [128, TK], F32, kind="ExternalInput").ap()
    out_d = nc.dram_tensor("outp", [L, D], BF16, kind="ExternalOutput").ap()

    with tile.TileContext(nc) as tc:
        _trace(nc, tc, xT_d, wT_d, negc_d, owT_d, maskf_d, out_d)
    nc.compile()
    return nc


def _trace(nc, tc, xT_d, wT_d, negc_d, owT_d, maskf_d, out_d):
    import contextlib
    ctx = contextlib.ExitStack()
    with ctx:
        pers = ctx.enter_context(tc.tile_pool(name="pers", bufs=1))
        tmp = ctx.enter_context(tc.tile_pool(name="tmp", bufs=2))
        ld = ctx.enter_context(tc.tile_pool(name="ld", bufs=1))

        # ---- persistent tiles ----
        xTb = pers.tile([128, KC + 1, L], BF16)       # chunks 0-7 x, 8 = mu row
        wb = pers.tile([128, KC, 768], BF16)
        negcb = pers.tile([1, 768], BF16)
        owTb = pers.tile([64, 4, D], BF16)
        maskf = pers.tile([128, TK], F32)
        mask16 = pers.tile([128, TK], BF16)
        rstd_bc = pers.tile([128, TT, 512], F32)      # rstd broadcast, query-major
        svec = pers.tile([128, TK], F32)              # rstd*mask, token-major
        ones_col = pers.tile([128, 1], BF16)
        ident = pers.tile([128, 128], F32)
        qT = [pers.tile([128, L], BF16, tag=f"qT{g}", name=f"qT{g}") for g in range(2)]
        kT = [pers.tile([128, L], BF16, tag=f"kT{g}", name=f"kT{g}") for g in range(2)]
        vaug = [pers.tile([128, TK, 65], BF16, tag=f"vaug{h}", name=f"vaug{h}") for h in range(4)]
        ctxT = [pers.tile([64, L], BF16, tag=f"ctxT{h}", name=f"ctxT{h}") for h in range(4)]

        nc.gpsimd.memset(ones_col[:], 1.0)
        make_identity(nc, ident[:])
        nc.sync.dma_start(maskf[:], maskf_d[:])
        nc.vector.tensor_copy(mask16[:], maskf[:])
        negcb_f = ld.tile([1, 768], F32, tag="negf")
        nc.sync.dma_start(negcb_f[:], negc_d[:])
        nc.vector.tensor_copy(negcb[:], negcb_f[:])
        owT_f = ld.tile([64, 4, D], F32, tag="owf")
        nc.sync.dma_start(owT_f[:], owT_d[:])
        nc.scalar.copy(owTb[:], owT_f[:])

        # ---- load + cast x / weights, and LN stats, interleaved ----
        with tc.tile_pool(name="stps", bufs=8, space="PSUM") as stps:
            st_mu = [stps.tile([1, 512], F32, tag="st", name=f"stm{tt}")
                     for tt in range(TT)]
            st_sq = [stps.tile([1, 512], F32, tag="st", name=f"sts{tt}")
                     for tt in range(TT)]
            for kc in range(KC):
                xf = tmp.tile([128, L], F32, tag="xf")
                nc.sync.dma_start(xf[:], xT_d[kc * 128:(kc + 1) * 128, :])
                nc.vector.tensor_copy(xTb[:, kc, :], xf[:])
                wf = tmp.tile([128, 768], F32, tag="wf")
                nc.sync.dma_start(wf[:], wT_d[kc * 128:(kc + 1) * 128, :])
                nc.scalar.copy(wb[:, kc, :], wf[:])
                xsq = tmp.tile([128, L], BF16, tag="xsq")
                nc.scalar.square(xsq[:], xTb[:, kc, :])
                for tt in range(TT):
                    sl = xTb[:, kc, tt * 512:(tt + 1) * 512]
                    nc.tensor.matmul(st_mu[tt][:], ones_col[:], sl,
                                     start=(kc == 0), stop=(kc == KC - 1))
                    nc.tensor.matmul(st_sq[tt][:], ones_col[:],
                                     xsq[:, tt * 512:(tt + 1) * 512],
                                     start=(kc == 0), stop=(kc == KC - 1))
            for tt in range(TT):
                # mu row into augmented x chunk (bf16)
                nc.vector.tensor_scalar_mul(
                    xTb[0:1, KC, tt * 512:(tt + 1) * 512], st_mu[tt][:], 1.0 / D)
                # var/rstd on the 1-partition rows
                mu = tmp.tile([1, 512], F32, tag="mu")
                var = tmp.tile([1, 512], F32, tag="var")
                nc.vector.tensor_scalar_mul(mu[:], st_mu[tt][:], 1.0 / D)
                nc.vector.tensor_scalar_mul(var[:], st_sq[tt][:], 1.0 / D)
                musq = tmp.tile([1, 512], F32, tag="musq")
                nc.vector.tensor_tensor(musq[:], mu[:], mu[:], mybir.AluOpType.mult)
                nc.vector.tensor_tensor(var[:], var[:], musq[:], mybir.AluOpType.subtract)
                nc.vector.tensor_scalar_add(var[:], var[:], LN_EPS)
                nc.scalar.activation(var[:], var[:], mybir.ActivationFunctionType.Sqrt,
                                     bias=0.0)
                nc.vector.reciprocal(var[:], var[:])
                nc.gpsimd.partition_broadcast(rstd_bc[:, tt, :], var[:])

        # svec = token-major rstd * mask via PE transpose of rstd_bc slices
        with tc.tile_pool(name="tps", bufs=2, space="PSUM") as tps:
            for tk in range(TK):
                tp = tps.tile([128, 128], F32)
                sl = rstd_bc[:, tk // 4, (tk % 4) * 128:(tk % 4) * 128 + 128]
                nc.tensor.transpose(tp[:], sl, ident[:])
                nc.vector.tensor_copy(svec[:, tk:tk + 1], tp[:, 0:1])
        nc.vector.tensor_tensor(svec[:], svec[:], maskf[:], mybir.AluOpType.mult)

        # ---- QKV projection ----
        with tc.tile_pool(name="qkps", bufs=3, space="PSUM") as qkps, \
             tc.tile_pool(name="vps", bufs=2, space="PSUM") as vps:
            # qT / kT: 4 col-groups of 128 (q pair0, q pair1, k pair0, k pair1)
            for tt in range(TT):
                for gi, (dest, cols) in enumerate(
                        [(qT[0], 0), (qT[1], 128), (kT[0], 256), (kT[1], 384)]):
                    ps = qkps.tile([128, 512], F32)
                    rhs = xTb[:, 0:KC, tt * 512:(tt + 1) * 512]
                    for kc in range(KC):
                        nc.tensor.matmul(ps[:], wb[:, kc, cols:cols + 128],
                                         rhs[:, kc, :], start=(kc == 0), stop=False)
                    nc.tensor.matmul(ps[:], negcb[0:1, cols:cols + 128],
                                     xTb[0:1, KC, tt * 512:(tt + 1) * 512],
                                     start=False, stop=True)
                    nc.vector.tensor_tensor(dest[:, tt * 512:(tt + 1) * 512],
                                            ps[:], rstd_bc[:, tt, :],
                                            mybir.AluOpType.mult)
            # v: token-major, all 4 heads (256 cols)
            for tk in range(TK):
                ps = vps.tile([128, 256], F32)
                for kc in range(KC):
                    nc.tensor.matmul(ps[:], xTb[:, kc, tk * 128:(tk + 1) * 128],
                                     wb[:, kc, 512:768], start=(kc == 0), stop=False)
                nc.tensor.matmul(ps[:], xTb[0:1, KC, tk * 128:(tk + 1) * 128],
                                 negcb[0:1, 512:768], start=False, stop=True)
                for h in range(4):
                    nc.vector.tensor_scalar_mul(vaug[h][:, tk, 0:64],
                                                ps[:, h * 64:(h + 1) * 64],
                                                svec[:, tk:tk + 1])
                    nc.vector.tensor_copy(vaug[h][:, tk, 64:65], mask16[:, tk:tk + 1])

        # ---- attention + output projection, interleaved per query tile ----
        with tc.tile_pool(name="scps", bufs=2, space="PSUM") as scps, \
             tc.tile_pool(name="pvps", bufs=3, space="PSUM") as pvps, \
             tc.tile_pool(name="ops", bufs=1, space="PSUM") as ops, \
             tc.tile_pool(name="et", bufs=4) as etp, \
             tc.tile_pool(name="rs", bufs=2) as rsp, \
             tc.tile_pool(name="osb", bufs=2) as osb:
            for qt in range(TT):        # query tile
                for g in range(2):      # head pair
                    pv = [pvps.tile([65, 512], F32, tag="pv", name=f"pv{hp}")
                          for hp in range(2)]
                    for quad in range(TK // 2):   # 2 key-chunks per step
                        sc = [scps.tile([128, 2, 512], F32, tag="sc", name=f"sc{hp}")
                              for hp in range(2)]
                        et = [etp.tile([128, 2, 512], BF16, tag="et", name=f"et{hp}")
                              for hp in range(2)]
                        for ci in range(2):
                            kc = quad * 2 + ci
                            for hp in range(2):
                                nc.tensor.matmul(
                                    sc[hp][:, ci, :],
                                    kT[g][hp * 64:hp * 64 + 64, kc * 128:(kc + 1) * 128],
                                    qT[g][hp * 64:hp * 64 + 64, qt * 512:(qt + 1) * 512],
                                    start=True, stop=True)
                        for hp in range(2):
                            nc.scalar.activation(et[hp][:], sc[hp][:],
                                                 mybir.ActivationFunctionType.Exp,
                                                 bias=0.0, scale=SCALE)
                        for ci in range(2):
                            kc = quad * 2 + ci
                            for hp in range(2):
                                h = g * 2 + hp
                                nc.tensor.matmul(pv[hp][:], vaug[h][:, kc, :],
                                                 et[hp][:, ci, :],
                                                 start=(kc == 0), stop=(kc == TK - 1))
                    for hp in range(2):
                        h = g * 2 + hp
                        rs = rsp.tile([1, 512], F32, tag="rs")
                        nc.vector.reciprocal(rs[:], pv[hp][64:65, :])
                        rb = rsp.tile([64, 512], F32, tag="rb")
                        nc.gpsimd.partition_broadcast(rb[:], rs[:])
                        nc.vector.tensor_tensor(ctxT[h][:, qt * 512:(qt + 1) * 512],
                                                pv[hp][0:64, :], rb[:],
                                                mybir.AluOpType.mult)
                # out-projection for this query tile's 4 token blocks
                for tj in range(4):
                    tk = qt * 4 + tj
                    ob = osb.tile([128, D], BF16, tag="ob")
                    for ncn in range(2):
                        ps = ops.tile([128, 512], F32, tag="op")
                        for h in range(4):
                            nc.tensor.matmul(ps[:], ctxT[h][:, tk * 128:(tk + 1) * 128],
                                             owTb[:, h, ncn * 512:(ncn + 1) * 512],
                                             start=(h == 0), stop=(h == 3))
                        nc.vector.tensor_copy(ob[:, ncn * 512:(ncn + 1) * 512], ps[:])
                    nc.sync.dma_start(out_d[tk * 128:(tk + 1) * 128, :], ob[:])


def _host_prep(inputs):
    x = np.asarray(inputs["x"], dtype=np.float32)
    attention_mask = np.asarray(inputs["attention_mask"])
    ln_w = np.asarray(inputs["ln_w"], dtype=np.float32)
    qkv_w = np.asarray(inputs["qkv_w"], dtype=np.float32)
    out_w = np.asarray(inputs["out_w"], dtype=np.float32)

    in_maps = []
    for c in range(NCORES):
        bc, g = divmod(c, 4)
        hs = g * HEADS_PER_CORE
        rows = np.concatenate([
            np.arange(hs * HD, (hs + 4) * HD),
            D + np.arange(hs * HD, (hs + 4) * HD),
            2 * D + np.arange(hs * HD, (hs + 4) * HD)])
        Wp = qkv_w[rows] * ln_w[None, :]                     # (768, D)
        wT = np.ascontiguousarray(Wp.T)                      # (D, 768)
        negc = np.ascontiguousarray(-Wp.sum(1)[None, :])     # (1, 768)
        ow = out_w[:, hs * HD:(hs + 4) * HD]                 # (D, 256)
        owT = np.ascontiguousarray(
            ow.T.reshape(4, HD, D).transpose(1, 0, 2))       # (64, 4, D)
        xT = np.ascontiguousarray(x[:, bc, :].T)             # (D, L)
        maskf = np.ascontiguousarray(
            attention_mask[bc].reshape(TK, 128).T.astype(np.float32))
        in_maps.append({"xT": xT, "wT": wT, "negc": negc,
                        "owT": owT, "maskf": maskf})
    return in_maps


def _gather(results, inputs):
    out_b = np.asarray(inputs["out_b"], dtype=np.float32)
    out = np.zeros((L, B, D), dtype=np.float32)
    for c in range(NCORES):
        bc = c // 4
        out[:, bc, :] += results[c]["outp"].astype(np.float32)
    out += out_b[None, None, :]
    return out


def kernel(**inputs):
    if "nc" not in _CACHE:
        _CACHE["nc"] = _build_nc()
    nc = _CACHE["nc"]
    in_maps = _host_prep(inputs)
    res = bass_utils.run_bass_kernel_spmd(nc, in_maps, core_ids=list(range(NCORES)))
    return _gather(res.results, inputs)


# revision 21
# speedup vs baseline: 1.1968x; 1.1968x over previous
"""Trainium2 Bass kernel for nn_Attention_927712935992.

Fused LayerNorm + QKV projection + masked softmax attention + output
projection, sharded over 8 NeuronCores: core c handles batch c//4 and
heads [4*(c%4), 4*(c%4)+4) of 16.  Weights are replicated (sliced per
core); the (B*H, L, L) score tensor is split along its first axis.

Per-core dataflow (all matmul operands bf16, fp32 PSUM accumulation):
  - host pre-transposes x -> xT (D, L) and the weight slices
  - device: token mean via ones-matmul rows; var/rstd from mu/musq rows
  - LN is folded algebraically into the QKV matmul:
      q = rstd * [ (x @ W'^T) - mu * rowsum(W') ],  W' = ln_w * W
    using an augmented K=1 contraction row (mu) with lhsT = -rowsum(W').
  - scores^T (keys on partitions) per (head, qtile): 2-head row-packed
    K=64 matmuls; exp on ScalarE (scale=1/8) straight from PSUM.
  - masking is free: v rows and the appended ones-column of v are
    multiplied by the 0/1 key mask, so masked keys drop out of both the
    context accumulation and the softmax denominator.
  - context^T = [v|mask] ^T E^T with a rowsum row; normalize by the
    reciprocal rowsum (gpsimd partition-broadcast) into bf16 context.
  - out partial = context^T.T @ out_w_slice^T accumulated over 4 heads.
Host sums the 4 per-batch partials and adds out_b.
"""

import numpy as np

import concourse.bass as bass
import concourse.tile as tile
from concourse import bacc
from concourse import mybir
from concourse import bass_utils
from concourse.masks import make_identity

L, B, D, H, HD = 2048, 2, 1024, 16, 64
NCORES = 8
HEADS_PER_CORE = 4
KC = D // 128            # 8 contraction chunks of 128
TT = 4                   # query tiles of 512
TK = L // 128            # 16 key/token chunks of 128
F32 = mybir.dt.float32
BF16 = mybir.dt.bfloat16
SCALE = 0.125            # 1/sqrt(64)
LN_EPS = 1e-12

_CACHE = {}


def _build_nc():
    nc = bacc.Bacc("TRN2", target_bir_lowering=False, debug=False)

    xT_d = nc.dram_tensor("xT", [D, L], F32, kind="ExternalInput").ap()
    wT_d = nc.dram_tensor("wT", [D, 768], F32, kind="ExternalInput").ap()
    negc_d = nc.dram_tensor("negc", [1, 768], F32, kind="ExternalInput").ap()
    owT_d = nc.dram_tensor("owT", [64, 4, D], F32, kind="ExternalInput").ap()
    maskf_d = nc.dram_tensor("maskf", [128, TK], F32, kind="ExternalInput").ap()
    out_d = nc.dram_tensor("outp", [L, D], BF16, kind="ExternalOutput").ap()

    with tile.TileContext(nc) as tc:
        _trace(nc, tc, xT_d, wT_d, negc_d, owT_d, maskf_d, out_d)
    nc.compile()
    return nc


def _trace(nc, tc, xT_d, wT_d, negc_d, owT_d, maskf_d, out_d):
    import contextlib
    ctx = contextlib.ExitStack()
    with ctx:
        pers = ctx.enter_context(tc.tile_pool(name="pers", bufs=1))
        tmp = ctx.enter_context(tc.tile_pool(name="tmp", bufs=2))
        ld = ctx.enter_context(tc.tile_pool(name="ld", bufs=1))

        # ---- persistent tiles ----
        xTb = pers.tile([128, KC + 1, L], BF16)       # chunks 0-7 x, 8 = mu row
        wb = pers.tile([128, KC, 768], BF16)
        negcb = pers.tile([1, 768], BF16)
        owTb = pers.tile([64, 4, D], BF16)
        maskf = pers.tile([128, TK], F32)
        mask16 = pers.tile([128, TK], BF16)
        rstd_bc = pers.tile([128, TT, 512], F32)      # rstd broadcast, query-major
        svec = pers.tile([128, TK], F32)              # rstd*mask, token-major
        ones_col = pers.tile([128, 1], BF16)
        ident = pers.tile([128, 128], F32)
        qT = [pers.tile([128, L], BF16, tag=f"qT{g}", name=f"qT{g}") for g in range(2)]
        # kT zero-padded to full 128 contraction rows per head: head hp's
        # 64 dims live at partitions 64*hp..64*hp+64, the other half is 0,
        # so score matmuls contract over K=128 (full PE array).
        kTz = [pers.tile([128, L], BF16, tag=f"kTz{g}{hp}", name=f"kTz{g}{hp}")
               for g in range(2) for hp in range(2)]
        vaug = [pers.tile([128, TK, 65], BF16, tag=f"vaug{h}", name=f"vaug{h}") for h in range(4)]
        ctxT = [pers.tile([64, L], BF16, tag=f"ctxT{h}", name=f"ctxT{h}") for h in range(4)]

        nc.gpsimd.memset(ones_col[:], 1.0)
        for z in range(4):
            nc.vector.memset(kTz[z][:], 0.0)
        make_identity(nc, ident[:])
        nc.sync.dma_start(maskf[:], maskf_d[:])
        nc.vector.tensor_copy(mask16[:], maskf[:])
        negcb_f = ld.tile([1, 768], F32, tag="negf")
        nc.sync.dma_start(negcb_f[:], negc_d[:])
        nc.vector.tensor_copy(negcb[:], negcb_f[:])
        owT_f = ld.tile([64, 4, D], F32, tag="owf")
        nc.sync.dma_start(owT_f[:], owT_d[:])
        nc.scalar.copy(owTb[:], owT_f[:])

        # ---- load + cast x / weights, and LN stats, interleaved ----
        with tc.tile_pool(name="stps", bufs=8, space="PSUM") as stps:
            st_mu = [stps.tile([1, 512], F32, tag="st", name=f"stm{tt}")
                     for tt in range(TT)]
            st_sq = [stps.tile([1, 512], F32, tag="st", name=f"sts{tt}")
                     for tt in range(TT)]
            for kc in range(KC):
                xf = tmp.tile([128, L], F32, tag="xf")
                nc.sync.dma_start(xf[:], xT_d[kc * 128:(kc + 1) * 128, :])
                nc.vector.tensor_copy(xTb[:, kc, :], xf[:])
                wf = tmp.tile([128, 768], F32, tag="wf")
                nc.sync.dma_start(wf[:], wT_d[kc * 128:(kc + 1) * 128, :])
                nc.scalar.copy(wb[:, kc, :], wf[:])
                xsq = tmp.tile([128, L], BF16, tag="xsq")
                nc.scalar.square(xsq[:], xTb[:, kc, :])
                for tt in range(TT):
                    sl = xTb[:, kc, tt * 512:(tt + 1) * 512]
                    nc.tensor.matmul(st_mu[tt][:], ones_col[:], sl,
                                     start=(kc == 0), stop=(kc == KC - 1))
                    nc.tensor.matmul(st_sq[tt][:], ones_col[:],
                                     xsq[:, tt * 512:(tt + 1) * 512],
                                     start=(kc == 0), stop=(kc == KC - 1))
            for tt in range(TT):
                # mu row into augmented x chunk (bf16)
                nc.vector.tensor_scalar_mul(
                    xTb[0:1, KC, tt * 512:(tt + 1) * 512], st_mu[tt][:], 1.0 / D)
                # var/rstd on the 1-partition rows
                mu = tmp.tile([1, 512], F32, tag="mu")
                var = tmp.tile([1, 512], F32, tag="var")
                nc.vector.tensor_scalar_mul(mu[:], st_mu[tt][:], 1.0 / D)
                nc.vector.tensor_scalar_mul(var[:], st_sq[tt][:], 1.0 / D)
                musq = tmp.tile([1, 512], F32, tag="musq")
                nc.vector.tensor_tensor(musq[:], mu[:], mu[:], mybir.AluOpType.mult)
                nc.vector.tensor_tensor(var[:], var[:], musq[:], mybir.AluOpType.subtract)
                nc.vector.tensor_scalar_add(var[:], var[:], LN_EPS)
                nc.scalar.activation(var[:], var[:], mybir.ActivationFunctionType.Sqrt,
                                     bias=0.0)
                nc.vector.reciprocal(var[:], var[:])
                nc.gpsimd.partition_broadcast(rstd_bc[:, tt, :], var[:])

        # svec = token-major rstd * mask via PE transpose of rstd_bc slices
        with tc.tile_pool(name="tps", bufs=2, space="PSUM") as tps:
            for tk in range(TK):
                tp = tps.tile([128, 128], F32)
                sl = rstd_bc[:, tk // 4, (tk % 4) * 128:(tk % 4) * 128 + 128]
                nc.tensor.transpose(tp[:], sl, ident[:])
                nc.vector.tensor_copy(svec[:, tk:tk + 1], tp[:, 0:1])
        nc.vector.tensor_tensor(svec[:], svec[:], maskf[:], mybir.AluOpType.mult)

        # ---- QKV projection ----
        with tc.tile_pool(name="qkps", bufs=3, space="PSUM") as qkps, \
             tc.tile_pool(name="vps", bufs=2, space="PSUM") as vps:
            # qT / kT: 4 col-groups of 128 (q pair0, q pair1, k pair0, k pair1)
            for tt in range(TT):
                for gi, cols in enumerate([0, 128, 256, 384]):
                    ps = qkps.tile([128, 512], F32)
                    rhs = xTb[:, 0:KC, tt * 512:(tt + 1) * 512]
                    for kc in range(KC):
                        nc.tensor.matmul(ps[:], wb[:, kc, cols:cols + 128],
                                         rhs[:, kc, :], start=(kc == 0), stop=False)
                    nc.tensor.matmul(ps[:], negcb[0:1, cols:cols + 128],
                                     xTb[0:1, KC, tt * 512:(tt + 1) * 512],
                                     start=False, stop=True)
                    sl = slice(tt * 512, (tt + 1) * 512)
                    if gi < 2:      # q pair gi
                        nc.vector.tensor_tensor(qT[gi][:, sl], ps[:],
                                                rstd_bc[:, tt, :],
                                                mybir.AluOpType.mult)
                    else:           # k pair g: split into zero-padded tiles
                        g = gi - 2
                        nc.vector.tensor_tensor(kTz[2 * g][0:64, sl], ps[0:64, :],
                                                rstd_bc[0:64, tt, :],
                                                mybir.AluOpType.mult)
                        nc.vector.tensor_tensor(kTz[2 * g + 1][64:128, sl],
                                                ps[64:128, :],
                                                rstd_bc[64:128, tt, :],
                                                mybir.AluOpType.mult)
            # v: token-major, all 4 heads (256 cols)
            for tk in range(TK):
                ps = vps.tile([128, 256], F32)
                for kc in range(KC):
                    nc.tensor.matmul(ps[:], xTb[:, kc, tk * 128:(tk + 1) * 128],
                                     wb[:, kc, 512:768], start=(kc == 0), stop=False)
                nc.tensor.matmul(ps[:], xTb[0:1, KC, tk * 128:(tk + 1) * 128],
                                 negcb[0:1, 512:768], start=False, stop=True)
                for h in range(4):
                    nc.vector.tensor_scalar_mul(vaug[h][:, tk, 0:64],
                                                ps[:, h * 64:(h + 1) * 64],
                                                svec[:, tk:tk + 1])
                    nc.vector.tensor_copy(vaug[h][:, tk, 64:65], mask16[:, tk:tk + 1])

        # ---- attention + output projection, interleaved per query tile ----
        with tc.tile_pool(name="scps", bufs=2, space="PSUM") as scps, \
             tc.tile_pool(name="pvps", bufs=3, space="PSUM") as pvps, \
             tc.tile_pool(name="ops", bufs=1, space="PSUM") as ops, \
             tc.tile_pool(name="et", bufs=4) as etp, \
             tc.tile_pool(name="rs", bufs=2) as rsp, \
             tc.tile_pool(name="osb", bufs=2) as osb:
            for qt in range(TT):        # query tile
                for g in range(2):      # head pair
                    pv = [pvps.tile([65, 512], F32, tag="pv", name=f"pv{hp}")
                          for hp in range(2)]
                    for quad in range(TK // 2):   # 2 key-chunks per step
                        sc = [scps.tile([128, 2, 512], F32, tag="sc", name=f"sc{hp}")
                              for hp in range(2)]
                        et = [etp.tile([128, 2, 512], BF16, tag="et", name=f"et{hp}")
                              for hp in range(2)]
                        for ci in range(2):
                            kc = quad * 2 + ci
                            for hp in range(2):
                                nc.tensor.matmul(
                                    sc[hp][:, ci, :],
                                    kTz[2 * g + hp][:, kc * 128:(kc + 1) * 128],
                                    qT[g][:, qt * 512:(qt + 1) * 512],
                                    start=True, stop=True)
                        for hp in range(2):
                            nc.scalar.activation(et[hp][:], sc[hp][:],
                                                 mybir.ActivationFunctionType.Exp,
                                                 bias=0.0, scale=SCALE)
                        for ci in range(2):
                            kc = quad * 2 + ci
                            for hp in range(2):
                                h = g * 2 + hp
                                nc.tensor.matmul(pv[hp][:], vaug[h][:, kc, :],
                                                 et[hp][:, ci, :],
                                                 start=(kc == 0), stop=(kc == TK - 1))
                    for hp in range(2):
                        h = g * 2 + hp
                        rs = rsp.tile([1, 512], F32, tag="rs")
                        nc.vector.reciprocal(rs[:], pv[hp][64:65, :])
                        rb = rsp.tile([64, 512], F32, tag="rb")
                        nc.gpsimd.partition_broadcast(rb[:], rs[:])
                        nc.vector.tensor_tensor(ctxT[h][:, qt * 512:(qt + 1) * 512],
                                                pv[hp][0:64, :], rb[:],
                                                mybir.AluOpType.mult)
                # out-projection for this query tile's 4 token blocks
                for tj in range(4):
                    tk = qt * 4 + tj
                    ob = osb.tile([128, D], BF16, tag="ob")
                    for ncn in range(2):
                        ps = ops.tile([128, 512], F32, tag="op")
                        for h in range(4):
                            nc.tensor.matmul(ps[:], ctxT[h][:, tk * 128:(tk + 1) * 128],
                                             owTb[:, h, ncn * 512:(ncn + 1) * 512],
                                             start=(h == 0), stop=(h == 3))
                        nc.vector.tensor_copy(ob[:, ncn * 512:(ncn + 1) * 512], ps[:])
                    nc.sync.dma_start(out_d[tk * 128:(tk + 1) * 128, :], ob[:])


def _host_prep(inputs):
    x = np.asarray(inputs["x"], dtype=np.float32)
    attention_mask = np.asarray(inputs["attention_mask"])
    ln_w = np.asarray(inputs["ln_w"], dtype=np.float32)
    qkv_w = np.asarray(inputs["qkv_w"], dtype=np.float32)
    out_w = np.asarray(inputs["out_w"], dtype=np.float32)

    in_maps = []
    for c in range(NCORES):
        bc, g = divmod(c, 4)
        hs = g * HEADS_PER_CORE
        rows = np.concatenate([
            np.arange(hs * HD, (hs + 4) * HD),
            D + np.arange(hs * HD, (hs + 4) * HD),
            2 * D + np.arange(hs * HD, (hs + 4) * HD)])
        Wp = qkv_w[rows] * ln_w[None, :]                     # (768, D)
        wT = np.ascontiguousarray(Wp.T)                      # (D, 768)
        negc = np.ascontiguousarray(-Wp.sum(1)[None, :])     # (1, 768)
        ow = out_w[:, hs * HD:(hs + 4) * HD]                 # (D, 256)
        owT = np.ascontiguousarray(
            ow.T.reshape(4, HD, D).transpose(1, 0, 2))       # (64, 4, D)
        xT = np.ascontiguousarray(x[:, bc, :].T)             # (D, L)
        maskf = np.ascontiguousarray(
            attention_mask[bc].reshape(TK, 128).T.astype(np.float32))
        in_maps.append({"xT": xT, "wT": wT, "negc": negc,
                        "owT": owT, "maskf": maskf})
    return in_maps


def _gather(results, inputs):
    out_b = np.asarray(inputs["out_b"], dtype=np.float32)
    out = np.zeros((L, B, D), dtype=np.float32)
    for c in range(NCORES):
        bc = c // 4
        out[:, bc, :] += results[c]["outp"].astype(np.float32)
    out += out_b[None, None, :]
    return out


def kernel(**inputs):
    if "nc" not in _CACHE:
        _CACHE["nc"] = _build_nc()
    nc = _CACHE["nc"]
    in_maps = _host_prep(inputs)
    res = bass_utils.run_bass_kernel_spmd(nc, in_maps, core_ids=list(range(NCORES)))
    return _gather(res.results, inputs)


# revision 28
# speedup vs baseline: 1.3325x; 1.1134x over previous
"""Trainium2 Bass kernel for nn_Attention_927712935992.

Fused LayerNorm + QKV projection + masked softmax attention + output
projection, sharded over 8 NeuronCores: core c handles batch c//4 and
heads [4*(c%4), 4*(c%4)+4) of 16.  Weights are replicated (sliced per
core); the (B*H, L, L) score tensor is split along its first axis.

Per-core dataflow (all matmul operands bf16, fp32 PSUM accumulation):
  - host pre-transposes x -> xT (D, L) and the weight slices
  - device: token mean via ones-matmul rows; var/rstd from mu/musq rows
  - LN is folded algebraically into the QKV matmul:
      q = rstd * [ (x @ W'^T) - mu * rowsum(W') ],  W' = ln_w * W
    using an augmented K=1 contraction row (mu) with lhsT = -rowsum(W').
  - scores^T (keys on partitions) per (head, qtile): 2-head row-packed
    K=64 matmuls; exp on ScalarE (scale=1/8) straight from PSUM.
  - masking is free: v rows and the appended ones-column of v are
    multiplied by the 0/1 key mask, so masked keys drop out of both the
    context accumulation and the softmax denominator.
  - context^T = [v|mask] ^T E^T with a rowsum row; normalize by the
    reciprocal rowsum (gpsimd partition-broadcast) into bf16 context.
  - out partial = context^T.T @ out_w_slice^T accumulated over 4 heads.
Host sums the 4 per-batch partials and adds out_b.
"""

import numpy as np

import concourse.bass as bass
import concourse.tile as tile
from concourse import bacc
from concourse import mybir
from concourse import bass_utils
from concourse.masks import make_identity

L, B, D, H, HD = 2048, 2, 1024, 16, 64
NCORES = 8
HEADS_PER_CORE = 4
KC = D // 128            # 8 contraction chunks of 128
TT = 4                   # query tiles of 512
TK = L // 128            # 16 key/token chunks of 128
F32 = mybir.dt.float32
BF16 = mybir.dt.bfloat16
FP8 = mybir.dt.float8e4
FP8_SCORES = False
SCALE = 0.125            # 1/sqrt(64)
LN_EPS = 1e-12

_CACHE = {}


def _build_nc():
    nc = bacc.Bacc("TRN2", target_bir_lowering=False, debug=False)

    xT_d = nc.dram_tensor("xT", [D, L], F32, kind="ExternalInput").ap()
    wT_d = nc.dram_tensor("wT", [D, 768], F32, kind="ExternalInput").ap()
    negc_d = nc.dram_tensor("negc", [1, 768], F32, kind="ExternalInput").ap()
    owT_d = nc.dram_tensor("owT", [128, 2, D], F32, kind="ExternalInput").ap()
    maskf_d = nc.dram_tensor("maskf", [128, TK], F32, kind="ExternalInput").ap()
    out_d = nc.dram_tensor("outp", [L, D], BF16, kind="ExternalOutput").ap()

    with tile.TileContext(nc) as tc:
        _trace(nc, tc, xT_d, wT_d, negc_d, owT_d, maskf_d, out_d)
    nc.compile()
    return nc


def _trace(nc, tc, xT_d, wT_d, negc_d, owT_d, maskf_d, out_d):
    import contextlib
    ctx = contextlib.ExitStack()
    with ctx:
        pers = ctx.enter_context(tc.tile_pool(name="pers", bufs=1))
        tmp = ctx.enter_context(tc.tile_pool(name="tmp", bufs=2))
        ld = ctx.enter_context(tc.tile_pool(name="ld", bufs=1))

        # ---- persistent tiles ----
        xTb = pers.tile([128, KC + 1, L], BF16)       # chunks 0-7 x, 8 = mu row
        wb = pers.tile([128, KC, 768], BF16)
        negcb = pers.tile([1, 768], BF16)
        maskf = pers.tile([128, TK], F32)
        mask16 = pers.tile([128, TK], BF16)
        rstd_bc = pers.tile([128, TT, 512], F32)      # rstd broadcast, query-major
        svec = pers.tile([128, TK], F32)              # rstd*mask, token-major
        ones_col = pers.tile([128, 1], BF16)
        ident = pers.tile([128, 128], F32)
        # fp8 operands for DoubleRow matmuls.  qT8/kT8z carry a dummy
        # second K-partner plane (index 1) that is zeroed so the packed
        # contraction K=(d, j) reduces to the real K=128.  kT8z is also
        # zero-padded outside the head's 64 dims (full-array matmuls keep
        # the PE clock governor warm).
        if FP8_SCORES:
            qT8 = [pers.tile([128, 2, L], FP8, tag=f"qT8{g}", name=f"qT8{g}")
                   for g in range(2)]
            kT8z = [pers.tile([128, 2, L], FP8, tag=f"kT8z{g}{hp}",
                              name=f"kT8z{g}{hp}")
                    for g in range(2) for hp in range(2)]
        else:
            qT8 = [pers.tile([128, 1, L], BF16, tag=f"qT8{g}", name=f"qT8{g}")
                   for g in range(2)]
            kT8z = [pers.tile([128, 1, L], BF16, tag=f"kT8z{g}{hp}",
                              name=f"kT8z{g}{hp}")
                    for g in range(2) for hp in range(2)]
        # vaug layouts give head0 of a pair [v | mask | 0...] (ctx at psum
        # rows 0-63, rowsum at 64) and head1 [mask | 0... | v] (rowsum at
        # row 0, ctx at rows 64-127), so both heads' context lands
        # lane-aligned in one stacked (128, L) tile per pair -> the output
        # projection contracts K=128 over a head pair in one matmul.
        vaugA = [pers.tile([128, TK, 65], BF16, tag=f"vaugA{g}", name=f"vaugA{g}")
                 for g in range(2)]
        vaugB = [pers.tile([128, TK, 128], BF16, tag=f"vaugB{g}", name=f"vaugB{g}")
                 for g in range(2)]
        ctxS = [pers.tile([128, L], BF16, tag=f"ctxS{g}", name=f"ctxS{g}")
                for g in range(2)]
        owTS = pers.tile([128, 2, D], BF16)

        nc.gpsimd.memset(ones_col[:], 1.0)
        for z in range(4):
            nc.gpsimd.memset(kT8z[z][:], 0.0)
        for g in range(2):
            if FP8_SCORES:
                nc.gpsimd.memset(qT8[g][:, 1, :], 0.0)
            nc.gpsimd.memset(vaugB[g][:], 0.0)
        make_identity(nc, ident[:])
        nc.sync.dma_start(maskf[:], maskf_d[:])
        nc.vector.tensor_copy(mask16[:], maskf[:])
        negcb_f = ld.tile([1, 768], F32, tag="negf")
        nc.sync.dma_start(negcb_f[:], negc_d[:])
        nc.vector.tensor_copy(negcb[:], negcb_f[:])
        owT_f = ld.tile([128, 2, D], F32, tag="owf")
        nc.sync.dma_start(owT_f[:], owT_d[:])
        nc.scalar.copy(owTS[:], owT_f[:])

        # ---- load + cast x / weights, and LN stats, interleaved ----
        with tc.tile_pool(name="stps", bufs=8, space="PSUM") as stps:
            st_mu = [stps.tile([1, 512], F32, tag="st", name=f"stm{tt}")
                     for tt in range(TT)]
            st_sq = [stps.tile([1, 512], F32, tag="st", name=f"sts{tt}")
                     for tt in range(TT)]
            for kc in range(KC):
                xf = tmp.tile([128, L], F32, tag="xf")
                nc.sync.dma_start(xf[:], xT_d[kc * 128:(kc + 1) * 128, :])
                nc.vector.tensor_copy(xTb[:, kc, :], xf[:])
                wf = tmp.tile([128, 768], F32, tag="wf")
                nc.sync.dma_start(wf[:], wT_d[kc * 128:(kc + 1) * 128, :])
                nc.scalar.copy(wb[:, kc, :], wf[:])
                xsq = tmp.tile([128, L], BF16, tag="xsq")
                nc.scalar.square(xsq[:], xTb[:, kc, :])
                for tt in range(TT):
                    sl = xTb[:, kc, tt * 512:(tt + 1) * 512]
                    nc.tensor.matmul(st_mu[tt][:], ones_col[:], sl,
                                     start=(kc == 0), stop=(kc == KC - 1))
                    nc.tensor.matmul(st_sq[tt][:], ones_col[:],
                                     xsq[:, tt * 512:(tt + 1) * 512],
                                     start=(kc == 0), stop=(kc == KC - 1))
            for tt in range(TT):
                # mu row into augmented x chunk (bf16)
                nc.vector.tensor_scalar_mul(
                    xTb[0:1, KC, tt * 512:(tt + 1) * 512], st_mu[tt][:], 1.0 / D)
                # var/rstd on the 1-partition rows
                mu = tmp.tile([1, 512], F32, tag="mu")
                var = tmp.tile([1, 512], F32, tag="var")
                nc.vector.tensor_scalar_mul(mu[:], st_mu[tt][:], 1.0 / D)
                nc.vector.tensor_scalar_mul(var[:], st_sq[tt][:], 1.0 / D)
                musq = tmp.tile([1, 512], F32, tag="musq")
                nc.vector.tensor_tensor(musq[:], mu[:], mu[:], mybir.AluOpType.mult)
                nc.vector.tensor_tensor(var[:], var[:], musq[:], mybir.AluOpType.subtract)
                nc.vector.tensor_scalar_add(var[:], var[:], LN_EPS)
                # rstd = exp(-0.5*ln(var)) — Ln and Exp share one ACT table set
                nc.scalar.activation(var[:], var[:], mybir.ActivationFunctionType.Ln,
                                     bias=0.0)
                nc.scalar.activation(var[:], var[:], mybir.ActivationFunctionType.Exp,
                                     bias=0.0, scale=-0.5)
                nc.gpsimd.partition_broadcast(rstd_bc[:, tt, :], var[:])

        # svec = token-major rstd * mask via PE transpose of rstd_bc slices
        with tc.tile_pool(name="tps", bufs=2, space="PSUM") as tps:
            for tk in range(TK):
                tp = tps.tile([128, 128], F32)
                sl = rstd_bc[:, tk // 4, (tk % 4) * 128:(tk % 4) * 128 + 128]
                nc.tensor.transpose(tp[:], sl, ident[:])
                nc.vector.tensor_copy(svec[:, tk:tk + 1], tp[:, 0:1])
        nc.vector.tensor_tensor(svec[:], svec[:], maskf[:], mybir.AluOpType.mult)

        # ---- QKV projection ----
        with tc.tile_pool(name="qkps", bufs=3, space="PSUM") as qkps, \
             tc.tile_pool(name="vps", bufs=2, space="PSUM") as vps:
            # qT / kT: 4 col-groups of 128 (q pair0, q pair1, k pair0, k pair1)
            for tt in range(TT):
                for gi, cols in enumerate([0, 128, 256, 384]):
                    ps = qkps.tile([128, 512], F32)
                    rhs = xTb[:, 0:KC, tt * 512:(tt + 1) * 512]
                    for kc in range(KC):
                        nc.tensor.matmul(ps[:], wb[:, kc, cols:cols + 128],
                                         rhs[:, kc, :], start=(kc == 0), stop=False)
                    nc.tensor.matmul(ps[:], negcb[0:1, cols:cols + 128],
                                     xTb[0:1, KC, tt * 512:(tt + 1) * 512],
                                     start=False, stop=True)
                    sl = slice(tt * 512, (tt + 1) * 512)
                    if gi < 2:      # q pair gi
                        nc.vector.tensor_tensor(qT8[gi][:, 0, sl], ps[:],
                                                rstd_bc[:, tt, :],
                                                mybir.AluOpType.mult)
                    else:           # k pair g: split into zero-padded tiles
                        g = gi - 2
                        nc.vector.tensor_tensor(kT8z[2 * g][0:64, 0, sl],
                                                ps[0:64, :],
                                                rstd_bc[0:64, tt, :],
                                                mybir.AluOpType.mult)
                        nc.vector.tensor_tensor(kT8z[2 * g + 1][64:128, 0, sl],
                                                ps[64:128, :],
                                                rstd_bc[64:128, tt, :],
                                                mybir.AluOpType.mult)
            # v: token-major, all 4 heads (256 cols)
            for tk in range(TK):
                ps = vps.tile([128, 256], F32)
                for kc in range(KC):
                    nc.tensor.matmul(ps[:], xTb[:, kc, tk * 128:(tk + 1) * 128],
                                     wb[:, kc, 512:768], start=(kc == 0), stop=False)
                nc.tensor.matmul(ps[:], xTb[0:1, KC, tk * 128:(tk + 1) * 128],
                                 negcb[0:1, 512:768], start=False, stop=True)
                for g in range(2):
                    nc.vector.tensor_scalar_mul(vaugA[g][:, tk, 0:64],
                                                ps[:, g * 128:g * 128 + 64],
                                                svec[:, tk:tk + 1])
                    nc.vector.tensor_copy(vaugA[g][:, tk, 64:65], mask16[:, tk:tk + 1])
                    nc.vector.tensor_scalar_mul(vaugB[g][:, tk, 64:128],
                                                ps[:, g * 128 + 64:g * 128 + 128],
                                                svec[:, tk:tk + 1])
                    nc.vector.tensor_copy(vaugB[g][:, tk, 0:1], mask16[:, tk:tk + 1])

        # ---- attention + output projection, interleaved per query tile ----
        with tc.tile_pool(name="scps", bufs=2, space="PSUM") as scps, \
             tc.tile_pool(name="pvps", bufs=4, space="PSUM") as pvps, \
             tc.tile_pool(name="et", bufs=4) as etp, \
             tc.tile_pool(name="rs", bufs=2) as rsp, \
             tc.tile_pool(name="osb", bufs=2) as osb:
            for qt in range(TT):        # query tile
                for g in range(2):      # head pair
                    pv = [pvps.tile([128, 512], F32, tag="pv", name=f"pv{hp}")
                          for hp in range(2)]
                    for quad in range(TK // 2):   # 2 key-chunks per step
                        sc = [scps.tile([128, 2, 512], F32, tag="sc", name=f"sc{hp}")
                              for hp in range(2)]
                        et = [etp.tile([128, 2, 512], BF16, tag="et", name=f"et{hp}")
                              for hp in range(2)]
                        pm = mybir.MatmulPerfMode.DoubleRow if FP8_SCORES else None
                        for ci in range(2):
                            kc = quad * 2 + ci
                            for hp in range(2):
                                nc.tensor.matmul(
                                    sc[hp][:, ci, :],
                                    kT8z[2 * g + hp][:, :, kc * 128:(kc + 1) * 128],
                                    qT8[g][:, :, qt * 512:(qt + 1) * 512],
                                    start=True, stop=True,
                                    perf_mode=pm)
                        for hp in range(2):
                            nc.scalar.activation(et[hp][:], sc[hp][:],
                                                 mybir.ActivationFunctionType.Exp,
                                                 bias=0.0, scale=SCALE)
                        for ci in range(2):
                            kc = quad * 2 + ci
                            nc.tensor.matmul(pv[0][0:65, :],
                                             vaugA[g][:, kc, :], et[0][:, ci, :],
                                             start=(kc == 0), stop=(kc == TK - 1))
                            nc.tensor.matmul(pv[1][:],
                                             vaugB[g][:, kc, :], et[1][:, ci, :],
                                             start=(kc == 0), stop=(kc == TK - 1))
                    sl = slice(qt * 512, (qt + 1) * 512)
                    # cross-base DVE writes (32-aligned bases are legal) put
                    # each rowsum reciprocal at a tile's partition 0: the
                    # gpsimd broadcast only reads physical partition 0
                    rsA = rsp.tile([1, 512], F32, tag="rsA")
                    nc.vector.reciprocal(rsA[:], pv[0][64:65, :])
                    rsB = rsp.tile([1, 512], F32, tag="rsB")
                    nc.vector.reciprocal(rsB[:], pv[1][0:1, :])
                    rb0 = rsp.tile([128, 512], F32, tag="rb0")
                    nc.gpsimd.partition_broadcast(rb0[:], rsA[:])
                    nc.vector.tensor_tensor(ctxS[g][0:64, sl], pv[0][0:64, :],
                                            rb0[0:64, :], mybir.AluOpType.mult)
                    rb1 = rsp.tile([128, 512], F32, tag="rb1")
                    nc.gpsimd.partition_broadcast(rb1[:], rsB[:])
                    nc.vector.tensor_tensor(ctxS[g][64:128, sl], pv[1][64:128, :],
                                            rb1[64:128, :], mybir.AluOpType.mult)
                # out-projection for this query tile's 4 token blocks
                for tj in range(4):
                    tk = qt * 4 + tj
                    ob = osb.tile([128, D], BF16, tag="ob")
                    for ncn in range(2):
                        ps = scps.tile([128, 2, 512], F32, tag="sc", name="op")
                        for gg in range(2):
                            nc.tensor.matmul(
                                ps[:, 0, :],
                                ctxS[gg][:, tk * 128:(tk + 1) * 128],
                                owTS[:, gg, ncn * 512:(ncn + 1) * 512],
                                start=(gg == 0), stop=(gg == 1))
                        nc.vector.tensor_copy(ob[:, ncn * 512:(ncn + 1) * 512],
                                              ps[:, 0, :])
                    nc.sync.dma_start(out_d[tk * 128:(tk + 1) * 128, :], ob[:])


def _host_prep(inputs):
    x = np.asarray(inputs["x"], dtype=np.float32)
    attention_mask = np.asarray(inputs["attention_mask"])
    ln_w = np.asarray(inputs["ln_w"], dtype=np.float32)
    qkv_w = np.asarray(inputs["qkv_w"], dtype=np.float32)
    out_w = np.asarray(inputs["out_w"], dtype=np.float32)

    in_maps = []
    for c in range(NCORES):
        bc, g = divmod(c, 4)
        hs = g * HEADS_PER_CORE
        rows = np.concatenate([
            np.arange(hs * HD, (hs + 4) * HD),
            D + np.arange(hs * HD, (hs + 4) * HD),
            2 * D + np.arange(hs * HD, (hs + 4) * HD)])
        Wp = qkv_w[rows] * ln_w[None, :]                     # (768, D)
        wT = np.ascontiguousarray(Wp.T)                      # (D, 768)
        negc = np.ascontiguousarray(-Wp.sum(1)[None, :])     # (1, 768)
        ow = out_w[:, hs * HD:(hs + 4) * HD]                 # (D, 256)
        owT = np.ascontiguousarray(
            ow.T.reshape(2, 2, HD, D).transpose(1, 2, 0, 3)
            .reshape(128, 2, D))                             # [hp*64+e, g, n]
        xT = np.ascontiguousarray(x[:, bc, :].T)             # (D, L)
        maskf = np.ascontiguousarray(
            attention_mask[bc].reshape(TK, 128).T.astype(np.float32))
        in_maps.append({"xT": xT, "wT": wT, "negc": negc,
                        "owT": owT, "maskf": maskf})
    return in_maps


def _gather(results, inputs):
    out_b = np.asarray(inputs["out_b"], dtype=np.float32)
    out = np.zeros((L, B, D), dtype=np.float32)
    for c in range(NCORES):
        bc = c // 4
        out[:, bc, :] += results[c]["outp"].astype(np.float32)
    out += out_b[None, None, :]
    return out


def kernel(**inputs):
    if "nc" not in _CACHE:
        _CACHE["nc"] = _build_nc()
    nc = _CACHE["nc"]
    in_maps = _host_prep(inputs)
    res = bass_utils.run_bass_kernel_spmd(nc, in_maps, core_ids=list(range(NCORES)))
    return _gather(res.results, inputs)


# revision 29
# speedup vs baseline: 1.3942x; 1.0463x over previous
"""Trainium2 Bass kernel for nn_Attention_927712935992.

Fused LayerNorm + QKV projection + masked softmax attention + output
projection, sharded over 8 NeuronCores: core c handles batch c//4 and
heads [4*(c%4), 4*(c%4)+4) of 16.  Weights are replicated (sliced per
core); the (B*H, L, L) score tensor is split along its first axis.

Per-core dataflow (all matmul operands bf16, fp32 PSUM accumulation):
  - host pre-transposes x -> xT (D, L) and the weight slices
  - device: token mean via ones-matmul rows; var/rstd from mu/musq rows
  - LN is folded algebraically into the QKV matmul:
      q = rstd * [ (x @ W'^T) - mu * rowsum(W') ],  W' = ln_w * W
    using an augmented K=1 contraction row (mu) with lhsT = -rowsum(W').
  - scores^T (keys on partitions) per (head, qtile): 2-head row-packed
    K=64 matmuls; exp on ScalarE (scale=1/8) straight from PSUM.
  - masking is free: v rows and the appended ones-column of v are
    multiplied by the 0/1 key mask, so masked keys drop out of both the
    context accumulation and the softmax denominator.
  - context^T = [v|mask] ^T E^T with a rowsum row; normalize by the
    reciprocal rowsum (gpsimd partition-broadcast) into bf16 context.
  - out partial = context^T.T @ out_w_slice^T accumulated over 4 heads.
Host sums the 4 per-batch partials and adds out_b.
"""

import numpy as np

import concourse.bass as bass
import concourse.tile as tile
from concourse import bacc
from concourse import mybir
from concourse import bass_utils
from concourse.masks import make_identity

L, B, D, H, HD = 2048, 2, 1024, 16, 64
NCORES = 8
HEADS_PER_CORE = 4
KC = D // 128            # 8 contraction chunks of 128
TT = 4                   # query tiles of 512
TK = L // 128            # 16 key/token chunks of 128
F32 = mybir.dt.float32
BF16 = mybir.dt.bfloat16
FP8 = mybir.dt.float8e4
FP8_SCORES = False
SCALE = 0.125            # 1/sqrt(64)
LN_EPS = 1e-12

_CACHE = {}


def _build_nc():
    nc = bacc.Bacc("TRN2", target_bir_lowering=False, debug=False)

    xT_d = nc.dram_tensor("xT", [D, L], F32, kind="ExternalInput").ap()
    wT_d = nc.dram_tensor("wT", [D, 768], F32, kind="ExternalInput").ap()
    negc_d = nc.dram_tensor("negc", [1, 768], F32, kind="ExternalInput").ap()
    owT_d = nc.dram_tensor("owT", [128, 2, D], F32, kind="ExternalInput").ap()
    maskf_d = nc.dram_tensor("maskf", [128, TK], F32, kind="ExternalInput").ap()
    out_d = nc.dram_tensor("outp", [L, D], BF16, kind="ExternalOutput").ap()

    with tile.TileContext(nc) as tc:
        _trace(nc, tc, xT_d, wT_d, negc_d, owT_d, maskf_d, out_d)
    nc.compile()
    return nc


def _trace(nc, tc, xT_d, wT_d, negc_d, owT_d, maskf_d, out_d):
    import contextlib
    ctx = contextlib.ExitStack()
    with ctx:
        pers = ctx.enter_context(tc.tile_pool(name="pers", bufs=1))
        tmp = ctx.enter_context(tc.tile_pool(name="tmp", bufs=2))
        ld = ctx.enter_context(tc.tile_pool(name="ld", bufs=1))

        # ---- persistent tiles ----
        xTb = pers.tile([128, KC + 1, L], BF16)       # chunks 0-7 x, 8 = mu row
        wb = pers.tile([128, KC, 768], BF16)
        negcb = pers.tile([1, 768], BF16)
        maskf = pers.tile([128, TK], F32)
        mask16 = pers.tile([128, TK], BF16)
        rstd_bc = pers.tile([128, TT, 512], F32)      # rstd broadcast, query-major
        svec = pers.tile([128, TK], F32)              # rstd*mask, token-major
        ones_col = pers.tile([128, 1], BF16)
        ident = pers.tile([128, 128], F32)
        # fp8 operands for DoubleRow matmuls.  qT8/kT8z carry a dummy
        # second K-partner plane (index 1) that is zeroed so the packed
        # contraction K=(d, j) reduces to the real K=128.  kT8z is also
        # zero-padded outside the head's 64 dims (full-array matmuls keep
        # the PE clock governor warm).
        if FP8_SCORES:
            qT8 = [pers.tile([128, 2, L], FP8, tag=f"qT8{g}", name=f"qT8{g}")
                   for g in range(2)]
            kT8z = [pers.tile([128, 2, L], FP8, tag=f"kT8z{g}{hp}",
                              name=f"kT8z{g}{hp}")
                    for g in range(2) for hp in range(2)]
        else:
            qT8 = [pers.tile([128, 1, L], BF16, tag=f"qT8{g}", name=f"qT8{g}")
                   for g in range(2)]
            kT8z = [pers.tile([128, 1, L], BF16, tag=f"kT8z{g}{hp}",
                              name=f"kT8z{g}{hp}")
                    for g in range(2) for hp in range(2)]
        # vaug layouts give head0 of a pair [v | mask | 0...] (ctx at psum
        # rows 0-63, rowsum at 64) and head1 [mask | 0... | v] (rowsum at
        # row 0, ctx at rows 64-127), so both heads' context lands
        # lane-aligned in one stacked (128, L) tile per pair -> the output
        # projection contracts K=128 over a head pair in one matmul.
        vaugA = [pers.tile([128, TK, 65], BF16, tag=f"vaugA{g}", name=f"vaugA{g}")
                 for g in range(2)]
        vaugB = [pers.tile([128, TK, 128], BF16, tag=f"vaugB{g}", name=f"vaugB{g}")
                 for g in range(2)]
        ctxS = [pers.tile([128, L], BF16, tag=f"ctxS{g}", name=f"ctxS{g}")
                for g in range(2)]
        owTS = pers.tile([128, 2, D], BF16)

        nc.gpsimd.memset(ones_col[:], 1.0)
        for g in range(2):
            nc.vector.memset(kT8z[2 * g][64:128, :, :], 0.0)
            nc.vector.memset(kT8z[2 * g + 1][0:64, :, :], 0.0)
            if FP8_SCORES:
                nc.gpsimd.memset(qT8[g][:, 1, :], 0.0)
                nc.gpsimd.memset(kT8z[2 * g][0:64, 1, :], 0.0)
                nc.gpsimd.memset(kT8z[2 * g + 1][64:128, 1, :], 0.0)
            nc.vector.memset(vaugB[g][:, :, 1:64], 0.0)
        make_identity(nc, ident[:])
        nc.sync.dma_start(maskf[:], maskf_d[:])
        nc.vector.tensor_copy(mask16[:], maskf[:])
        negcb_f = ld.tile([1, 768], F32, tag="negf")
        nc.sync.dma_start(negcb_f[:], negc_d[:])
        nc.vector.tensor_copy(negcb[:], negcb_f[:])
        owT_f = ld.tile([128, 2, D], F32, tag="owf")
        nc.sync.dma_start(owT_f[:], owT_d[:])
        nc.scalar.copy(owTS[:], owT_f[:])

        # ---- load + cast x / weights, and LN stats, interleaved ----
        with tc.tile_pool(name="stps", bufs=8, space="PSUM") as stps:
            st_mu = [stps.tile([1, 512], F32, tag="st", name=f"stm{tt}")
                     for tt in range(TT)]
            st_sq = [stps.tile([1, 512], F32, tag="st", name=f"sts{tt}")
                     for tt in range(TT)]
            for kc in range(KC):
                xf = tmp.tile([128, L], F32, tag="xf")
                nc.sync.dma_start(xf[:], xT_d[kc * 128:(kc + 1) * 128, :])
                nc.vector.tensor_copy(xTb[:, kc, :], xf[:])
                wf = tmp.tile([128, 768], F32, tag="wf")
                nc.sync.dma_start(wf[:], wT_d[kc * 128:(kc + 1) * 128, :])
                nc.scalar.copy(wb[:, kc, :], wf[:])
                xsq = tmp.tile([128, L], BF16, tag="xsq")
                nc.scalar.square(xsq[:], xTb[:, kc, :])
                for tt in range(TT):
                    sl = xTb[:, kc, tt * 512:(tt + 1) * 512]
                    nc.tensor.matmul(st_mu[tt][:], ones_col[:], sl,
                                     start=(kc == 0), stop=(kc == KC - 1))
                    nc.tensor.matmul(st_sq[tt][:], ones_col[:],
                                     xsq[:, tt * 512:(tt + 1) * 512],
                                     start=(kc == 0), stop=(kc == KC - 1))
            for tt in range(TT):
                # mu row into augmented x chunk (bf16)
                nc.vector.tensor_scalar_mul(
                    xTb[0:1, KC, tt * 512:(tt + 1) * 512], st_mu[tt][:], 1.0 / D)
                # var/rstd on the 1-partition rows
                mu = tmp.tile([1, 512], F32, tag="mu")
                var = tmp.tile([1, 512], F32, tag="var")
                nc.vector.tensor_scalar_mul(mu[:], st_mu[tt][:], 1.0 / D)
                nc.vector.tensor_scalar_mul(var[:], st_sq[tt][:], 1.0 / D)
                musq = tmp.tile([1, 512], F32, tag="musq")
                nc.vector.tensor_tensor(musq[:], mu[:], mu[:], mybir.AluOpType.mult)
                nc.vector.tensor_tensor(var[:], var[:], musq[:], mybir.AluOpType.subtract)
                nc.vector.tensor_scalar_add(var[:], var[:], LN_EPS)
                # rstd = exp(-0.5*ln(var)) — Ln and Exp share one ACT table set
                nc.scalar.activation(var[:], var[:], mybir.ActivationFunctionType.Ln,
                                     bias=0.0)
                nc.scalar.activation(var[:], var[:], mybir.ActivationFunctionType.Exp,
                                     bias=0.0, scale=-0.5)
                nc.gpsimd.partition_broadcast(rstd_bc[:, tt, :], var[:])

        # svec = token-major rstd * mask via PE transpose of rstd_bc slices
        with tc.tile_pool(name="tps", bufs=2, space="PSUM") as tps:
            for tk in range(TK):
                tp = tps.tile([128, 128], F32)
                sl = rstd_bc[:, tk // 4, (tk % 4) * 128:(tk % 4) * 128 + 128]
                nc.tensor.transpose(tp[:], sl, ident[:])
                nc.vector.tensor_copy(svec[:, tk:tk + 1], tp[:, 0:1])
        nc.vector.tensor_tensor(svec[:], svec[:], maskf[:], mybir.AluOpType.mult)

        # ---- QKV projection ----
        with tc.tile_pool(name="qkps", bufs=3, space="PSUM") as qkps, \
             tc.tile_pool(name="vps", bufs=2, space="PSUM") as vps:
            # qT / kT: 4 col-groups of 128 (q pair0, q pair1, k pair0, k pair1)
            for tt in range(TT):
                for gi, cols in enumerate([0, 128, 256, 384]):
                    ps = qkps.tile([128, 512], F32)
                    rhs = xTb[:, 0:KC, tt * 512:(tt + 1) * 512]
                    for kc in range(KC):
                        nc.tensor.matmul(ps[:], wb[:, kc, cols:cols + 128],
                                         rhs[:, kc, :], start=(kc == 0), stop=False)
                    nc.tensor.matmul(ps[:], negcb[0:1, cols:cols + 128],
                                     xTb[0:1, KC, tt * 512:(tt + 1) * 512],
                                     start=False, stop=True)
                    sl = slice(tt * 512, (tt + 1) * 512)
                    if gi < 2:      # q pair gi
                        nc.vector.tensor_tensor(qT8[gi][:, 0, sl], ps[:],
                                                rstd_bc[:, tt, :],
                                                mybir.AluOpType.mult)
                    else:           # k pair g: split into zero-padded tiles
                        g = gi - 2
                        nc.vector.tensor_tensor(kT8z[2 * g][0:64, 0, sl],
                                                ps[0:64, :],
                                                rstd_bc[0:64, tt, :],
                                                mybir.AluOpType.mult)
                        nc.vector.tensor_tensor(kT8z[2 * g + 1][64:128, 0, sl],
                                                ps[64:128, :],
                                                rstd_bc[64:128, tt, :],
                                                mybir.AluOpType.mult)
            # v: token-major, all 4 heads (256 cols)
            for tk in range(TK):
                ps = vps.tile([128, 256], F32)
                for kc in range(KC):
                    nc.tensor.matmul(ps[:], xTb[:, kc, tk * 128:(tk + 1) * 128],
                                     wb[:, kc, 512:768], start=(kc == 0), stop=False)
                nc.tensor.matmul(ps[:], xTb[0:1, KC, tk * 128:(tk + 1) * 128],
                                 negcb[0:1, 512:768], start=False, stop=True)
                for g in range(2):
                    nc.vector.tensor_scalar_mul(vaugA[g][:, tk, 0:64],
                                                ps[:, g * 128:g * 128 + 64],
                                                svec[:, tk:tk + 1])
                    nc.vector.tensor_copy(vaugA[g][:, tk, 64:65], mask16[:, tk:tk + 1])
                    nc.vector.tensor_scalar_mul(vaugB[g][:, tk, 64:128],
                                                ps[:, g * 128 + 64:g * 128 + 128],
                                                svec[:, tk:tk + 1])
                    nc.vector.tensor_copy(vaugB[g][:, tk, 0:1], mask16[:, tk:tk + 1])

        # ---- attention + output projection, interleaved per query tile ----
        with tc.tile_pool(name="scps", bufs=2, space="PSUM") as scps, \
             tc.tile_pool(name="pvps", bufs=4, space="PSUM") as pvps, \
             tc.tile_pool(name="et", bufs=4) as etp, \
             tc.tile_pool(name="rs", bufs=2) as rsp, \
             tc.tile_pool(name="osb", bufs=2) as osb:
            def outproj(qt):
                for tj in range(4):
                    tk = qt * 4 + tj
                    ob = osb.tile([128, D], BF16, tag="ob", name="ob")
                    for ncn in range(2):
                        ps = scps.tile([128, 2, 512], F32, tag="sc", name="op")
                        for gg in range(2):
                            nc.tensor.matmul(
                                ps[:, 0, :],
                                ctxS[gg][:, tk * 128:(tk + 1) * 128],
                                owTS[:, gg, ncn * 512:(ncn + 1) * 512],
                                start=(gg == 0), stop=(gg == 1))
                        nc.vector.tensor_copy(ob[:, ncn * 512:(ncn + 1) * 512],
                                              ps[:, 0, :])
                    nc.sync.dma_start(out_d[tk * 128:(tk + 1) * 128, :], ob[:])

            for qt in range(TT):        # query tile
                for g in range(2):      # head pair
                    pv = [pvps.tile([128, 512], F32, tag="pv", name=f"pv{hp}")
                          for hp in range(2)]
                    for quad in range(TK // 2):   # 2 key-chunks per step
                        sc = [scps.tile([128, 2, 512], F32, tag="sc", name=f"sc{hp}")
                              for hp in range(2)]
                        et = [etp.tile([128, 2, 512], BF16, tag="et", name=f"et{hp}")
                              for hp in range(2)]
                        pm = mybir.MatmulPerfMode.DoubleRow if FP8_SCORES else None
                        for ci in range(2):
                            kc = quad * 2 + ci
                            for hp in range(2):
                                nc.tensor.matmul(
                                    sc[hp][:, ci, :],
                                    kT8z[2 * g + hp][:, :, kc * 128:(kc + 1) * 128],
                                    qT8[g][:, :, qt * 512:(qt + 1) * 512],
                                    start=True, stop=True,
                                    perf_mode=pm)
                        for hp in range(2):
                            nc.scalar.activation(et[hp][:], sc[hp][:],
                                                 mybir.ActivationFunctionType.Exp,
                                                 bias=0.0, scale=SCALE)
                        for ci in range(2):
                            kc = quad * 2 + ci
                            nc.tensor.matmul(pv[0][0:65, :],
                                             vaugA[g][:, kc, :], et[0][:, ci, :],
                                             start=(kc == 0), stop=(kc == TK - 1))
                            nc.tensor.matmul(pv[1][:],
                                             vaugB[g][:, kc, :], et[1][:, ci, :],
                                             start=(kc == 0), stop=(kc == TK - 1))
                    sl = slice(qt * 512, (qt + 1) * 512)
                    # cross-base DVE writes (32-aligned bases are legal) put
                    # each rowsum reciprocal at a tile's partition 0: the
                    # gpsimd broadcast only reads physical partition 0
                    rsA = rsp.tile([1, 512], F32, tag="rsA")
                    nc.vector.reciprocal(rsA[:], pv[0][64:65, :])
                    rsB = rsp.tile([1, 512], F32, tag="rsB")
                    nc.vector.reciprocal(rsB[:], pv[1][0:1, :])
                    rb0 = rsp.tile([128, 512], F32, tag="rb0")
                    nc.gpsimd.partition_broadcast(rb0[:], rsA[:])
                    nc.vector.tensor_tensor(ctxS[g][0:64, sl], pv[0][0:64, :],
                                            rb0[0:64, :], mybir.AluOpType.mult)
                    rb1 = rsp.tile([128, 512], F32, tag="rb1")
                    nc.gpsimd.partition_broadcast(rb1[:], rsB[:])
                    nc.vector.tensor_tensor(ctxS[g][64:128, sl], pv[1][64:128, :],
                                            rb1[64:128, :], mybir.AluOpType.mult)
                # out-projection delayed one tile so its ctx dependency is
                # long satisfied and PE never stalls on the epilogue chain
                if qt > 0:
                    outproj(qt - 1)
            outproj(TT - 1)


def _host_prep(inputs):
    x = np.asarray(inputs["x"], dtype=np.float32)
    attention_mask = np.asarray(inputs["attention_mask"])
    ln_w = np.asarray(inputs["ln_w"], dtype=np.float32)
    qkv_w = np.asarray(inputs["qkv_w"], dtype=np.float32)
    out_w = np.asarray(inputs["out_w"], dtype=np.float32)

    in_maps = []
    for c in range(NCORES):
        bc, g = divmod(c, 4)
        hs = g * HEADS_PER_CORE
        rows = np.concatenate([
            np.arange(hs * HD, (hs + 4) * HD),
            D + np.arange(hs * HD, (hs + 4) * HD),
            2 * D + np.arange(hs * HD, (hs + 4) * HD)])
        Wp = qkv_w[rows] * ln_w[None, :]                     # (768, D)
        wT = np.ascontiguousarray(Wp.T)                      # (D, 768)
        negc = np.ascontiguousarray(-Wp.sum(1)[None, :])     # (1, 768)
        ow = out_w[:, hs * HD:(hs + 4) * HD]                 # (D, 256)
        owT = np.ascontiguousarray(
            ow.T.reshape(2, 2, HD, D).transpose(1, 2, 0, 3)
            .reshape(128, 2, D))                             # [hp*64+e, g, n]
        xT = np.ascontiguousarray(x[:, bc, :].T)             # (D, L)
        maskf = np.ascontiguousarray(
            attention_mask[bc].reshape(TK, 128).T.astype(np.float32))
        in_maps.append({"xT": xT, "wT": wT, "negc": negc,
                        "owT": owT, "maskf": maskf})
    return in_maps


def _gather(results, inputs):
    out_b = np.asarray(inputs["out_b"], dtype=np.float32)
    out = np.zeros((L, B, D), dtype=np.float32)
    for c in range(NCORES):
        bc = c // 4
        out[:, bc, :] += results[c]["outp"].astype(np.float32)
    out += out_b[None, None, :]
    return out


def kernel(**inputs):
    if "nc" not in _CACHE:
        _CACHE["nc"] = _build_nc()
    nc = _CACHE["nc"]
    in_maps = _host_prep(inputs)
    res = bass_utils.run_bass_kernel_spmd(nc, in_maps, core_ids=list(range(NCORES)))
    return _gather(res.results, inputs)
